# revision 54
# baseline (speedup 1.0000x reference)
"""HANLayer (2x GATConv + semantic attention) Trainium2 Bass kernel, 8 cores.

Strategy v2: aggregate in x-space (512-wide) instead of h-space (2048-wide),
exploiting linearity of segment-sum: sum_e alpha_e (x[src_e] @ W) =
(sum_e alpha_e x[src_e]) @ W.  This removes the replicated full projection
(x @ [W1 W2], 21.5 GFLOP/core) of the v1 kernel entirely.

Per core: 10 dst-node blocks of 128; edges sorted by dst, chunked by 128.
Per-edge attention-logit scalars s=x@(W@a_src), d=x@(W@a_dst) are computed
once for all nodes ([NPAD, 4]), scattered to a 256B-row DRAM table, and
fetched per-edge with batched dma_gather (one instruction per block) along
with the x rows.  One-hot edge->dst matrices (host-built) turn the weighted
segment-sum into matmuls accumulating aggT = sum alpha_e x_e^T directly in
transposed orientation, so the post-aggregation projection needs no
transposes.  Semantic attention runs pipelined per block group.
"""
import os
import sys

for _p in ("/opt/trn_rl_repo", "/root/.axon_site/_ro/trn_rl_repo"):
    if os.path.isdir(_p) and _p not in sys.path:
        sys.path.insert(0, _p)

import numpy as np
import ml_dtypes

import concourse.bacc as bacc
import concourse.bass as bass
import concourse.mybir as mybir
import concourse.tile as tile
from concourse import bass_utils
from concourse.masks import make_identity

F32 = mybir.dt.float32
BF16 = mybir.dt.bfloat16
FP8 = mybir.dt.float8e4
I32 = mybir.dt.int32

N = 10000
E = 160000
IN_C = 512
OUT_C = 1024
NEG_SLOPE = 0.2
NCORES = 8
NPAD = 10240
NBLK = NPAD // 128      # 80 dst blocks total
BPC = 10                # dst blocks per core
NODES_PER_CORE = 1280
P = 128

AddOp = mybir.AluOpType.add
SubOp = mybir.AluOpType.subtract
MulOp = mybir.AluOpType.mult
MaxOp = mybir.AluOpType.max


def _host_prep(edge_index):
    """Sort edges by dst; build per-core per-chunk gather indices and one-hot
    edge<->dst matrices (both orientations) per 128-edge chunk.  Self loops
    are NOT in the edge list -- the kernel handles them via a diagonal-matmul
    path from the dst-block rows (halves nothing but saves 1 gather chunk and
    1280 gather descriptors per core)."""
    src = edge_index[0].astype(np.int64)
    dst = edge_index[1].astype(np.int64)
    order = np.argsort(dst, kind="stable")
    src_s = src[order]
    dst_s = dst[order]
    blk = dst_s // P
    counts = np.bincount(blk, minlength=NBLK)
    K = int(np.ceil(counts.max() / P))
    CE = K * P
    src32 = np.zeros((NCORES, P, BPC * K), np.int32)
    em = np.zeros((NCORES, BPC, P, CE), np.float32)
    emt = np.zeros((NCORES, BPC, P, CE), np.float32)
    bstart = np.searchsorted(blk, np.arange(NBLK + 1))
    for b in range(NBLK):
        core, bslot = divmod(b, BPC)
        lo, hi = bstart[b], bstart[b + 1]
        nb = hi - lo
        fs = np.zeros(CE, np.int64)
        fs[:nb] = src_s[lo:hi]
        ii = np.arange(nb)
        dloc = dst_s[lo:hi] - b * P
        em[core, bslot, ii % P, (ii // P) * P + dloc] = 1.0
        emt[core, bslot, dloc, (ii // P) * P + ii % P] = 1.0
        src32[core, :, bslot * K:(bslot + 1) * K] = fs.reshape(K, P).T
    dbidx = np.zeros((NCORES, P, BPC), np.int32)
    for core in range(NCORES):
        for b in range(BPC):
            dbidx[core, :, b] = (core * BPC + b) * P + np.arange(P)
    return K, src32, em, emt, dbidx


def _build_program(K, zero_bias=True, debug=False):
    CL = K * P          # padded edges per block
    nc = bacc.Bacc("TRN2", target_bir_lowering=False, debug=False,
                   enable_asserts=False, num_devices=NCORES)

    RW = IN_C + 4       # XS row: [x(512) | s1 s2 | d1 d2]
    X516 = nc.dram_tensor("X516", [NPAD, RW], BF16, kind="ExternalInput")
    XT = nc.dram_tensor("XT", [IN_C, NPAD], BF16, kind="ExternalInput")
    WT12 = nc.dram_tensor("WT12", [2 * OUT_C, IN_C], BF16,
                          kind="ExternalInput")
    # a-vectors repacked [128, 8*4]: col block k = A4[k*128:(k+1)*128, :]
    A4R = nc.dram_tensor("A4R", [P, 32], BF16, kind="ExternalInput")
    W12 = nc.dram_tensor("W12", [2 * IN_C, OUT_C], BF16, kind="ExternalInput")
    WP1DR = nc.dram_tensor("WP1DR", [4 * P, 2 * OUT_C], FP8,
                           kind="ExternalInput")
    WP2 = nc.dram_tensor("WP2", [OUT_C, OUT_C], BF16, kind="ExternalInput")
    B12 = nc.dram_tensor("B12", [2, OUT_C], F32, kind="ExternalInput")
    BP1C = nc.dram_tensor("BP1C", [P, 8], F32, kind="ExternalInput")
    PRA = nc.dram_tensor("PRA", [1, 1], F32, kind="ExternalInput")
    MSK = nc.dram_tensor("MSK", [1, 1], F32, kind="ExternalInput")
    EMB = nc.dram_tensor("EMB", [BPC, P, CL], BF16, kind="ExternalInput")
    EMTB = nc.dram_tensor("EMTB", [BPC, P, CL], BF16, kind="ExternalInput")
    SRC32 = nc.dram_tensor("SRC32", [P, BPC * K], I32, kind="ExternalInput")
    DBIDX = nc.dram_tensor("DBIDX", [P, BPC], I32, kind="ExternalInput")

    OUT = nc.dram_tensor("OUT", [NODES_PER_CORE, OUT_C], F32,
                         kind="ExternalOutput")

    # s/d scalars are scattered into the tail columns of X516 itself (the
    # runtime re-uploads inputs each call, so device-side mutation is safe)
    XS = X516
    ARIN = nc.dram_tensor("ARIN", [OUT_C], F32, kind="Internal")
    AROUT = nc.dram_tensor("AROUT", [OUT_C], F32, kind="Internal",
                           addr_space="Shared")
    ATTD = nc.dram_tensor("ATTD", [1, OUT_C], F32, kind="Internal")

    # semantic-attention accum slots per o2 (per-block wp1): blocks 0..7 ->
    # slots 0..7; block 8 cols [0:16) -> slot 8, [16:128) -> slot 9 (node
    # 1040 within a core is where valid nodes end on the masked core);
    # block 9 -> slot 10
    NSLOT = 11

    with tile.TileContext(nc) as tc:
        with tc.tile_pool(name="persist", bufs=1) as pp:
            # NOTE: the sync HWDGE ring is one FIFO shared by all 16 DMA
            # engines -- issue ONLY the critical-path loads (wtt, XT) early;
            # everything else is issued later in program order and/or gated
            # behind phase A via WAW pre-writes.
            b1b = pp.tile([P, OUT_C], F32, tag="b1b")
            b2b = pp.tile([P, OUT_C], F32, tag="b2b")
            bp1c = pp.tile([P, 8], F32, tag="bp1c")
            pa_col = pp.tile([P, 1], F32, tag="pa_col")
            msk_col = pp.tile([P, 1], F32, tag="msk_col")
            ones_bf = pp.tile([P, 1], BF16, tag="ones")
            nc.vector.memset(ones_bf[:], 1.0)
            ident = pp.tile([P, P], BF16, tag="ident")
            make_identity(nc, ident[:])
            sidx = pp.tile([P, BPC * K], I32, tag="sidx")
            nc.scalar.dma_start(sidx[:], SRC32.ap())
            dbix = pp.tile([P, BPC], I32, tag="dbix")
            nc.scalar.dma_start(dbix[:], DBIDX.ap())
            sAcc = pp.tile([P, 4 * NBLK], BF16, tag="sAcc")
            accT = pp.tile([P, 8 * NSLOT], F32, tag="accT")
            tbar = pp.tile([P, 8], F32, tag="tbar")
            h1st = pp.tile([P, BPC * OUT_C], BF16, tag="h1st")
            h2st = pp.tile([P, BPC * OUT_C], BF16, tag="h2st")
            w12t = [pp.tile([P, OUT_C], BF16, tag=f"w12_{i}", name=f"w12_{i}")
                    for i in range(8)]
            # Wp1 in fp8 DoubleRow-interleaved layout: wp1d[k2][p, t*1024 +
            # o2*128 + m] = Wp1[(2*k2+t)*128+p, o2*128+m]
            wp1d = [pp.tile([P, 2 * OUT_C], FP8, tag=f"wp1d_{k2}",
                            name=f"wp1d_{k2}")
                    for k2 in range(4)]

            # ============ Phase A: attention-logit scalars S256 ============
            with tc.tile_pool(name="pAsb", bufs=1) as spA, \
                 tc.tile_pool(name="pAps", bufs=1, space="PSUM") as psA:
                a4r = spA.tile([P, 32], BF16, tag="a4r")
                nc.scalar.dma_start(a4r[:], A4R.ap())
                wtt = [spA.tile([P, IN_C], BF16, tag=f"wt_{i}", name=f"wt_{i}")
                       for i in range(16)]
                # wtt on the scalar ring so the critical XT load heads the
                # sync ring and starts at t~=0
                for i in range(16):
                    nc.scalar.dma_start(wtt[i][:],
                                        WT12.ap()[i * P:(i + 1) * P, :])
                # wtilde[:, 0:2] = [W1@a_src1, W1@a_dst1] (cols s1,d1)
                # wtilde[:, 2:4] = [W2@a_src2, W2@a_dst2] (cols s2,d2)
                w4sb = spA.tile([P, 16], BF16, tag="w4sb")
                for ic in range(4):
                    wps1 = psA.tile([P, 2], F32, tag="wps1", bufs=2)
                    wps2 = psA.tile([P, 2], F32, tag="wps2", bufs=2)
                    for k in range(8):
                        nc.tensor.matmul(wps1[:],
                                         lhsT=wtt[k][:, ic * P:(ic + 1) * P],
                                         rhs=a4r[:, 4 * k:4 * k + 2],
                                         start=(k == 0), stop=(k == 7))
                    for k in range(8):
                        nc.tensor.matmul(wps2[:],
                                         lhsT=wtt[8 + k][:, ic * P:(ic + 1) * P],
                                         rhs=a4r[:, 4 * k + 2:4 * k + 4],
                                         start=(k == 0), stop=(k == 7))
                    nc.vector.tensor_copy(w4sb[:, ic * 4:ic * 4 + 2], wps1[:])
                    nc.vector.tensor_copy(w4sb[:, ic * 4 + 2:ic * 4 + 4],
                                          wps2[:])
                # XT loaded in two NPAD halves so S matmuls start early
                HB = NPAD // 2
                xtg = [[spA.tile([P, HB], BF16, tag=f"xtg{g}_{h}",
                                 name=f"xtg{g}_{h}")
                        for g in range(4)] for h in range(2)]
                for h in range(2):
                    for g in range(4):
                        nc.sync.dma_start(
                            xtg[h][g][:],
                            XT.ap()[g * P:(g + 1) * P, h * HB:(h + 1) * HB])
                for i in range(NBLK):
                    h, ih = divmod(i, NBLK // 2)
                    sps = psA.tile([P, 4], F32, tag="sps", bufs=2)
                    for g in range(4):
                        nc.tensor.matmul(sps[:],
                                         lhsT=xtg[h][g][:, ih * P:
                                                        (ih + 1) * P],
                                         rhs=w4sb[:, g * 4:(g + 1) * 4],
                                         start=(g == 0), stop=(g == 3))
                    # psum cols [s1,d1,s2,d2] -> XS tail order [s1,s2,d1,d2]
                    nc.vector.tensor_copy(sAcc[:, 4 * i:4 * i + 2],
                                          sps[:, 0::2])
                    nc.vector.tensor_copy(sAcc[:, 4 * i + 2:4 * i + 4],
                                          sps[:, 1::2])
                    # scatter the tails in 10-block groups so the (slow,
                    # 8B-burst) scatter DMA overlaps the XT load instead of
                    # serializing after it; alternate between the sync and
                    # scalar HWDGE rings to halve ring-head processing time
                    if (i + 1) % 10 == 0:
                        gsc = (i + 1) // 10 - 1
                        eng = nc.sync if gsc % 2 == 0 else nc.scalar
                        eng.dma_start(
                            bass.AP(XS, RW * P * 10 * gsc + IN_C,
                                    [[RW, P], [RW * P, 10], [1, 4]]),
                            sAcc[:, 40 * gsc:40 * (gsc + 1)].rearrange(
                                "p (i c) -> p i c", i=10))
                # weight tiles for later phases: WAW-gate each tile behind
                # the last S-matmul copy (pre-write one column), so their
                # DMAs cannot fire before phase A's critical XT load ends
                for i in range(8):
                    nc.vector.tensor_copy(w12t[i][:, 0:1], sAcc[:, 319:320])
                    nc.sync.dma_start(w12t[i][:],
                                      W12.ap()[i * P:(i + 1) * P, :])
                for k2 in range(4):
                    nc.vector.tensor_copy(wp1d[k2][:, 0:1], sAcc[:, 319:320])
                    nc.sync.dma_start(wp1d[k2][:],
                                      WP1DR.ap()[k2 * P:(k2 + 1) * P, :])
                # bias tiles: partition-broadcast via a rank-1 fp32 matmul
                # (a DMA broadcast of [P,1024] blocks the sync ring ~10us)
                b12t = spA.tile([1, 2 * OUT_C], F32, tag="b12t")
                nc.sync.dma_start(b12t[:],
                                  bass.AP(B12, 0, [[2 * OUT_C, 1],
                                                   [1, 2 * OUT_C]]))
                ones1p = spA.tile([1, P], F32, tag="ones1p")
                nc.vector.memset(ones1p[:], 1.0)
                for l, bb in enumerate([b1b, b2b]):
                    bbps = psA.tile([P, OUT_C], F32, tag="bbps", bufs=1)
                    for hh in range(2):
                        # separate psum banks: independent start/stop groups
                        nc.tensor.matmul(
                            bbps[:, hh * 512:(hh + 1) * 512],
                            lhsT=ones1p[:],
                            rhs=b12t[0:1, l * OUT_C + hh * 512:
                                     l * OUT_C + (hh + 1) * 512],
                            start=True, stop=True)
                    nc.vector.tensor_copy(bb[:], bbps[:])
                nc.sync.dma_start(bp1c[:], BP1C.ap())
                nc.sync.dma_start(pa_col[:], PRA.ap().to_broadcast((P, 1)))
                nc.sync.dma_start(msk_col[:], MSK.ap().to_broadcast((P, 1)))

            # ============ Phase B: aggregation + proj + semantic ============
            with tc.tile_pool(name="pBsb", bufs=1) as sp, \
                 tc.tile_pool(name="pBps", bufs=1, space="PSUM") as ps:
                aggT12 = ps.tile([P, 1024], F32, tag="agg")
                ph = ps.tile([P, OUT_C], F32, tag="ph")
                tpp = ps.tile([P, 8 * P], BF16, tag="tpp")
                pre = {}
                st = {}

                def prep_io_em(b):
                    """em/emT/d-row loads for block b (two blocks ahead)."""
                    dblkr = sp.tile([P, RW], BF16, tag="dblkr", bufs=4)
                    nc.gpsimd.indirect_dma_start(
                        out=dblkr[:], out_offset=None, in_=XS.ap(),
                        in_offset=bass.IndirectOffsetOnAxis(
                            ap=dbix[:, b:b + 1], axis=0))
                    emb = sp.tile([P, CL], BF16, tag="emb", bufs=4)
                    nc.sync.dma_start(emb[:], EMB.ap()[b])
                    emtb = sp.tile([P, CL], BF16, tag="emtb", bufs=4)
                    nc.sync.dma_start(emtb[:], EMTB.ap()[b])
                    pre[("em", b)] = (dblkr, emb, emtb)

                def prep_io_xg(b):
                    """Edge gathers for block b (one block ahead)."""
                    xg = sp.tile([P, K * RW], BF16, tag="xg", bufs=3)
                    for c in range(K):
                        nc.gpsimd.indirect_dma_start(
                            out=xg[:, c * RW:(c + 1) * RW], out_offset=None,
                            in_=XS.ap(),
                            in_offset=bass.IndirectOffsetOnAxis(
                                ap=sidx[:, b * K + c:b * K + c + 1], axis=0))
                    pre[("xg", b)] = xg

                def prep_compute(b):
                    """d logits (PE) + alpha tiles (DVE/ACT) for block b."""
                    dblkr, emb, emtb = pre.pop(("em", b))
                    xg = pre.pop(("xg", b))
                    dblk = sp.tile([P, 2], BF16, tag="dblk", bufs=2)
                    nc.vector.tensor_copy(dblk[:], dblkr[:, IN_C + 2:RW])
                    # self-loop logits from the dst-block row itself:
                    # e_self = s_l + d_l, w_self = exp(leakyrelu(e_self))
                    eself = sp.tile([P, 2], F32, tag="eself", bufs=2)
                    nc.vector.tensor_tensor(
                        out=eself[:], in0=dblkr[:, IN_C:IN_C + 2],
                        in1=dblkr[:, IN_C + 2:RW], op=AddOp)
                    lrs = sp.tile([P, 2], F32, tag="lrs", bufs=2)
                    nc.vector.scalar_tensor_tensor(
                        out=lrs[:], in0=eself[:], scalar=NEG_SLOPE,
                        in1=eself[:], op0=MulOp, op1=MaxOp)
                    alself = sp.tile([P, 2], F32, tag="alself", bufs=2)
                    nc.scalar.activation(alself[:], lrs[:],
                                         mybir.ActivationFunctionType.Exp)
                    # diagonal alpha matrices for the self-loop agg matmuls
                    diag1 = sp.tile([P, P], BF16, tag="diag1", bufs=2)
                    nc.vector.tensor_scalar_mul(diag1[:], ident[:],
                                                alself[:, 0:1])
                    diag2 = sp.tile([P, P], BF16, tag="diag2", bufs=2)
                    nc.vector.tensor_scalar_mul(diag2[:], ident[:],
                                                alself[:, 1:2])
                    pre[("self", b)] = (dblkr, alself, diag1, diag2)
                    # dd bank: group 1 = per-edge d logits (cols 0:2K);
                    # reused later by group 2 = denominators (cols 0:2)
                    dd = ps.tile([P, 512], F32, tag="dd", bufs=2)
                    for c in range(K):
                        nc.tensor.matmul(dd[:, 2 * c:2 * c + 2],
                                         lhsT=emtb[:, c * P:(c + 1) * P],
                                         rhs=dblk[:],
                                         start=(c == 0), stop=(c == K - 1))
                    e2 = sp.tile([P, 2 * K], F32, tag="e2", bufs=2)
                    nc.vector.tensor_tensor(
                        out=e2[:].rearrange("p (k t) -> p k t", k=K),
                        in0=xg[:].rearrange("p (k e) -> p k e", k=K)[
                            :, :, IN_C:IN_C + 2],
                        in1=dd[:, 0:2 * K].rearrange("p (k t) -> p k t", k=K),
                        op=AddOp)
                    lr = sp.tile([P, 2 * K], F32, tag="lr", bufs=2)
                    nc.vector.scalar_tensor_tensor(
                        out=lr[:], in0=e2[:], scalar=NEG_SLOPE, in1=e2[:],
                        op0=MulOp, op1=MaxOp)
                    al = sp.tile([P, 2 * K], BF16, tag="al", bufs=2)
                    nc.scalar.activation(al[:], lr[:],
                                         mybir.ActivationFunctionType.Exp)
                    pre[b] = (xg, emb, None, al, dd)

                def prep_alpha(b):
                    xg, emb, _, al, dd = pre[b]
                    # one broadcast-AP op: a12b[p, c, l, d] = em[p,c,d]*al[p,c,l]
                    a12b = sp.tile([P, K * 256], BF16, tag="a12b", bufs=2)
                    nc.vector.tensor_tensor(
                        out=a12b[:].rearrange("p (k t d) -> p k t d",
                                              k=K, t=2),
                        in0=emb[:].rearrange("p (k one d) -> p k one d",
                                             k=K, one=1).to_broadcast(
                                                 (P, K, 2, P)),
                        in1=al[:].rearrange("p (k t one) -> p k t one",
                                            k=K, t=2, one=1).to_broadcast(
                                                (P, K, 2, P)),
                        op=MulOp)
                    pre[b] = (xg, emb, a12b, al, dd)

                def agg_matmuls(b):
                    xg, emb, a12b, al, dd = pre[b]
                    dblkr, alself, diag1, diag2 = pre.pop(("self", b))
                    for c in range(K):
                        # start=True clears has_written for the WHOLE psum
                        # bank: exactly one start/stop per bank (j pairs
                        # share a 512-f32 bank); the self-loop diag matmuls
                        # below carry the stop
                        for j in range(4):
                            nc.tensor.matmul(
                                aggT12[:, j * 256:(j + 1) * 256],
                                lhsT=xg[:, c * RW + j * P:
                                        c * RW + (j + 1) * P],
                                rhs=a12b[:, c * 256:(c + 1) * 256],
                                start=(c == 0 and j % 2 == 0),
                                stop=False)
                        # den[dst, l] += sum_e em[e,dst] * alpha[e, l]
                        # (second sequential group in the dd bank)
                        nc.tensor.matmul(dd[:, 0:2],
                                         lhsT=emb[:, c * P:(c + 1) * P],
                                         rhs=al[:, 2 * c:2 * c + 2],
                                         start=(c == 0), stop=(c == K - 1))
                    # self-loop contribution: aggT[f, d] += w_self[d]*x[d, f]
                    for j in range(4):
                        nc.tensor.matmul(
                            aggT12[:, j * 256:j * 256 + P],
                            lhsT=dblkr[:, j * P:(j + 1) * P],
                            rhs=diag1[:], start=False, stop=False)
                        nc.tensor.matmul(
                            aggT12[:, j * 256 + P:(j + 1) * 256],
                            lhsT=dblkr[:, j * P:(j + 1) * P],
                            rhs=diag2[:], start=False, stop=(j % 2 == 1))
                    pre[("selfw", b)] = alself

                def agg_copies(b):
                    xg, emb, a12b, al, dd = pre.pop(b)
                    alself = pre.pop(("selfw", b))
                    a1sb = sp.tile([P, IN_C], BF16, tag="a1sb", bufs=2)
                    a2sb = sp.tile([P, IN_C], BF16, tag="a2sb", bufs=2)
                    nc.vector.tensor_copy(
                        a1sb[:].rearrange("p (j d) -> p j d", j=4),
                        aggT12[:].rearrange("p (j d) -> p j d",
                                            j=4)[:, :, 0:P])
                    nc.vector.tensor_copy(
                        a2sb[:].rearrange("p (j d) -> p j d", j=4),
                        aggT12[:].rearrange("p (j d) -> p j d",
                                            j=4)[:, :, P:256])
                    # denominator includes the self-loop weight
                    dsum = sp.tile([P, 2], F32, tag="dsum", bufs=2)
                    nc.vector.tensor_tensor(out=dsum[:], in0=dd[:, 0:2],
                                            in1=alself[:], op=AddOp)
                    rden = sp.tile([P, 2], F32, tag="rden", bufs=2)
                    nc.vector.reciprocal(rden[:], dsum[:])
                    st[b] = (a1sb, a2sb, rden)

                def stage2a(b):
                    a1sb, a2sb, rden = st.pop(b)
                    hcols = slice(b * OUT_C, (b + 1) * OUT_C)
                    for l, (asb, bb, hst) in enumerate(
                            [(a1sb, b1b, h1st), (a2sb, b2b, h2st)]):
                        for j in range(4):
                            for hh in range(2):
                                nc.tensor.matmul(
                                    ph[:, hh * 512:(hh + 1) * 512],
                                    lhsT=asb[:, j * P:(j + 1) * P],
                                    rhs=w12t[l * 4 + j][:, hh * 512:
                                                        (hh + 1) * 512],
                                    start=(j == 0), stop=(j == 3))
                        if zero_bias:
                            # fused on the scalar engine:
                            # prelu(ph*rden) == Prelu(in*scale), alpha=pa
                            nc.scalar.activation(
                                hst[:, hcols], ph[:],
                                mybir.ActivationFunctionType.Prelu,
                                scale=rden[:, l:l + 1],
                                alpha=pa_col[:, 0:1])
                        else:
                            hstage = sp.tile([P, OUT_C], BF16, tag="hstage",
                                             bufs=2)
                            nc.vector.scalar_tensor_tensor(
                                out=hstage[:], in0=ph[:],
                                scalar=rden[:, l:l + 1], in1=bb[:],
                                op0=MulOp, op1=AddOp)
                            # prelu(v) = max(a*v, v) for 0<=a<=1
                            nc.vector.scalar_tensor_tensor(
                                out=hst[:, hcols], in0=hstage[:],
                                scalar=pa_col[:, 0:1], in1=hstage[:],
                                op0=MulOp, op1=MaxOp)
                    hsum = sp.tile([P, OUT_C], BF16, tag="hsum", bufs=2)
                    nc.vector.tensor_tensor(out=hsum[:], in0=h1st[:, hcols],
                                            in1=h2st[:, hcols], op=AddOp)
                    st[("hsum", b)] = hsum

                def stage2b(b):
                    hsum = st.pop(("hsum", b))
                    htgt = sp.tile([P, 8 * P], FP8, tag="htg", bufs=2)
                    # 8 transposes packed into one bf16 psum bank: single
                    # accumulation group (one start/stop for the bank)
                    for q in range(8):
                        nc.tensor.matmul(tpp[:, q * P:(q + 1) * P],
                                         lhsT=hsum[:, q * P:(q + 1) * P],
                                         rhs=ident[:], is_transpose=True,
                                         start=(q == 0), stop=(q == 7))
                    nc.vector.tensor_copy(htgt[:], tpp[:])
                    st[("htgt", b)] = htgt

                def wp1_block(b):
                    # per-block semantic-attention partial sums: accum slot
                    # b for b<8; block 8 splits at col 16 (masked-core valid
                    # boundary) into slots 8/9; block 9 -> slot 10
                    htgt = st.pop(("htgt", b))
                    tps = ps.tile([P, P], F32, tag="tps", bufs=1)
                    ts = sp.tile([P, P], BF16, tag="ts", bufs=2)
                    for o2 in range(8):
                        for k2 in range(4):
                            # fp8 DoubleRow: 2 contraction k-tiles per matmul
                            nc.tensor.matmul(
                                tps[:],
                                lhsT=wp1d[k2][:].rearrange(
                                    "p (t o m) -> p t o m",
                                    t=2, o=8)[:, :, o2, :],
                                rhs=htgt[:, k2 * 256:(k2 + 1) * 256]
                                    .rearrange("p (t n) -> p t n", t=2),
                                start=(k2 == 0), stop=(k2 == 3),
                                perf_mode=mybir.MatmulPerfMode.DoubleRow)
                        if b < 8:
                            nc.scalar.activation(
                                ts[:], tps[:],
                                mybir.ActivationFunctionType.Tanh,
                                bias=bp1c[:, o2:o2 + 1],
                                accum_out=accT[:, b * 8 + o2:b * 8 + o2 + 1])
                        elif b == 8:
                            nc.scalar.activation(
                                ts[:, 0:16], tps[:, 0:16],
                                mybir.ActivationFunctionType.Tanh,
                                bias=bp1c[:, o2:o2 + 1],
                                accum_out=accT[:, 64 + o2:64 + o2 + 1])
                            nc.scalar.activation(
                                ts[:, 16:P], tps[:, 16:P],
                                mybir.ActivationFunctionType.Tanh,
                                bias=bp1c[:, o2:o2 + 1],
                                accum_out=accT[:, 72 + o2:72 + o2 + 1])
                        else:
                            nc.scalar.activation(
                                ts[:], tps[:],
                                mybir.ActivationFunctionType.Tanh,
                                bias=bp1c[:, o2:o2 + 1],
                                accum_out=accT[:, 80 + o2:80 + o2 + 1])

                # gate the em/emt DMA rings behind phase A: fill all 4 ring
                # slots with dummy tiles whose writer depends on the last
                # S-matmul copy, so the 11.8MB of one-hot loads can't float
                # to t=0 and steal bandwidth from the critical XT load
                for gi in range(4):
                    g_em = sp.tile([P, CL], BF16, tag="emb", bufs=4,
                                   name=f"gate_em{gi}")
                    nc.vector.tensor_copy(g_em[:, 0:1], sAcc[:, 319:320])
                    g_emt = sp.tile([P, CL], BF16, tag="emtb", bufs=4,
                                    name=f"gate_emt{gi}")
                    nc.vector.tensor_copy(g_emt[:, 0:1], sAcc[:, 319:320])
                prep_io_em(0)
                prep_io_em(1)
                prep_io_xg(0)
                for b in range(BPC + 2):
                    if b < BPC:
                        prep_compute(b)
                    if 2 <= b <= BPC + 1:
                        stage2b(b - 2)
                    if b < BPC:
                        prep_alpha(b)
                    if 1 <= b <= BPC:
                        agg_matmuls(b - 1)
                    # wp1 after the agg matmuls: its PSUM ping-pong stall
                    # (tps bufs=1) then sits off the critical PE path
                    if 2 <= b <= BPC + 1:
                        wp1_block(b - 2)
                    if b + 2 < BPC:
                        prep_io_em(b + 2)
                    if b + 1 < BPC:
                        prep_io_xg(b + 1)
                    if 1 <= b <= BPC:
                        agg_copies(b - 1)
                        # stage2a in the same iteration as its agg_copies:
                        # one pipeline stage less to flush after the last
                        # gather
                        stage2a(b - 1)

            # ================= tail: softmax + blend =================
            with tc.tile_pool(name="pTsb", bufs=1) as sp, \
                 tc.tile_pool(name="pTps", bufs=1, space="PSUM") as ps:
                # WP2 loads overlap the collective latency; WAW-gated on the
                # block-7 semantic accum slots so they fire near loop end
                # rather than floating to t=0
                wp2t = [sp.tile([P, OUT_C], BF16, tag=f"wp2_{k}",
                                name=f"wp2_{k}") for k in range(8)]
                for k in range(8):
                    nc.vector.tensor_copy(wp2t[k][:, 0:1], sAcc[:, 0:1])
                    nc.sync.dma_start(wp2t[k][:],
                                      WP2.ap()[k * P:(k + 1) * P, :])
                # tbar[o2] = sum(slots 0..8) + msk*(slot9 + slot10)
                tA = sp.tile([P, 8], F32, tag="tA", bufs=4)
                nc.vector.tensor_tensor(out=tA[:], in0=accT[:, 0:8],
                                        in1=accT[:, 8:16], op=AddOp)
                tB = sp.tile([P, 8], F32, tag="tB", bufs=4)
                nc.vector.tensor_tensor(out=tB[:], in0=accT[:, 16:24],
                                        in1=accT[:, 24:32], op=AddOp)
                tC = sp.tile([P, 8], F32, tag="tC", bufs=4)
                nc.vector.tensor_tensor(out=tC[:], in0=accT[:, 32:40],
                                        in1=accT[:, 40:48], op=AddOp)
                tD = sp.tile([P, 8], F32, tag="tD", bufs=4)
                nc.vector.tensor_tensor(out=tD[:], in0=accT[:, 48:56],
                                        in1=accT[:, 56:64], op=AddOp)
                tAB = sp.tile([P, 8], F32, tag="tAB", bufs=4)
                nc.vector.tensor_tensor(out=tAB[:], in0=tA[:], in1=tB[:],
                                        op=AddOp)
                tCD = sp.tile([P, 8], F32, tag="tCD", bufs=4)
                nc.vector.tensor_tensor(out=tCD[:], in0=tC[:], in1=tD[:],
                                        op=AddOp)
                t07 = sp.tile([P, 8], F32, tag="t07", bufs=4)
                nc.vector.tensor_tensor(out=t07[:], in0=tAB[:], in1=tCD[:],
                                        op=AddOp)
                t08 = sp.tile([P, 8], F32, tag="t08", bufs=4)
                nc.vector.tensor_tensor(out=t08[:], in0=t07[:],
                                        in1=accT[:, 64:72], op=AddOp)
                tU = sp.tile([P, 8], F32, tag="tU", bufs=4)
                nc.vector.tensor_tensor(out=tU[:], in0=accT[:, 72:80],
                                        in1=accT[:, 80:88], op=AddOp)
                nc.vector.scalar_tensor_tensor(
                    out=tbar[:], in0=tU[:], scalar=msk_col[:, 0:1],
                    in1=t08[:], op0=MulOp, op1=AddOp)
                arview = [[1, P], [P, 8]]
                nc.scalar.dma_start(bass.AP(ARIN, 0, arview), tbar[:])
                nc.gpsimd.collective_compute(
                    "AllReduce", AddOp,
                    replica_groups=[list(range(NCORES))],
                    ins=[ARIN.ap().opt()], outs=[AROUT.ap().opt()])
                tbm = sp.tile([P, 8], F32, tag="tbm")
                nc.sync.dma_start(tbm[:], bass.AP(AROUT, 0, arview))
                tbn = sp.tile([P, 8], BF16, tag="tbn")
                nc.vector.tensor_scalar_mul(tbn[:], tbm[:], 1.0 / N)
                pw = ps.tile([1, OUT_C], F32, tag="pw")
                for k in range(8):
                    nc.tensor.matmul(pw[:, 0:512], lhsT=tbn[:, k:k + 1],
                                     rhs=wp2t[k][:, 0:512], start=(k == 0),
                                     stop=(k == 7))
                    nc.tensor.matmul(pw[:, 512:1024], lhsT=tbn[:, k:k + 1],
                                     rhs=wp2t[k][:, 512:1024], start=(k == 0),
                                     stop=(k == 7))
                et = sp.tile([1, OUT_C], F32, tag="et")
                esum = sp.tile([1, 1], F32, tag="esum")
                nc.scalar.activation(et[:], pw[:],
                                     mybir.ActivationFunctionType.Exp,
                                     accum_out=esum[:])
                rs = sp.tile([1, 1], F32, tag="rs")
                nc.vector.reciprocal(rs[:], esum[:])
                att1 = sp.tile([1, OUT_C], BF16, tag="att1")
                nc.vector.tensor_scalar_mul(att1[:], et[:], rs[:, 0:1])
                # broadcast att across partitions via rank-1 matmul
                ones1r = sp.tile([1, P], BF16, tag="ones1r")
                nc.vector.memset(ones1r[:], 1.0)
                attps = ps.tile([P, OUT_C], F32, tag="attps")
                for hh in range(2):
                    nc.tensor.matmul(attps[:, hh * 512:(hh + 1) * 512],
                                     lhsT=ones1r[:],
                                     rhs=att1[:, hh * 512:(hh + 1) * 512])
                attbh = sp.tile([P, OUT_C], BF16, tag="attbh")
                nc.vector.tensor_copy(attbh[:], attps[:])
                for b in range(BPC):
                    hcols = slice(b * OUT_C, (b + 1) * OUT_C)
                    # h1-h2 on gpsimd (idle in the tail) to halve DVE work
                    d = sp.tile([P, OUT_C], BF16, tag="bd", bufs=2)
                    nc.gpsimd.tensor_tensor(out=d[:], in0=h1st[:, hcols],
                                            in1=h2st[:, hcols], op=SubOp)
                    m = sp.tile([P, OUT_C], BF16, tag="bm", bufs=2)
                    nc.vector.tensor_tensor(out=m[:], in0=d[:], in1=attbh[:],
                                            op=MulOp)
                    o = sp.tile([P, OUT_C], F32, tag="bo", bufs=3)
                    nc.vector.tensor_tensor(out=o[:], in0=m[:],
                                            in1=h2st[:, hcols], op=AddOp)
                    # alternate output writes across the sync and scalar
                    # HWDGE rings so the 5.2MB f32 writeback is not
                    # serialized on one ring
                    eng = nc.sync if b % 2 == 0 else nc.scalar
                    eng.dma_start(OUT.ap()[b * P:(b + 1) * P, :], o[:])

    nc.compile()
    return nc


_PROG_CACHE = {}


def _ensure_trace_support():
    """Install the missing antenv.axon_hooks NTFF shim so trace=True works."""
    import types
    try:
        from antenv import axon_hooks  # noqa: F401
        return True
    except ImportError:
        pass
    try:
        import antenv
        if "/root/.axon_site" not in sys.path:
            sys.path.append("/root/.axon_site")
        from trn_agent_boot.trn_boot import _ntff_profile_via_ctypes
        hook = _ntff_profile_via_ctypes("/opt/axon/libaxon_pjrt.so")
        if hook is None:
            return False
        mod = types.ModuleType("antenv.axon_hooks")
        mod._hook = hook
        mod.get_axon_ntff_profile_hook = lambda: mod._hook
        mod.set_axon_ntff_profile_hook = lambda h: setattr(mod, "_hook", h)
        sys.modules["antenv.axon_hooks"] = mod
        antenv.axon_hooks = mod
        bass_utils.upload_artifacts = lambda t: str(t)
        return True
    except Exception as e:  # noqa: BLE001
        print("trace support unavailable:", e)
        return False


def _get_program(K, zero_bias):
    key = (K, zero_bias)
    if key not in _PROG_CACHE:
        _PROG_CACHE[key] = _build_program(K, zero_bias)
    return _PROG_CACHE[key]


def _run(inputs, trace=False, tmpdir=None):
    x = np.asarray(inputs["x"], np.float32)
    edge_index = np.asarray(inputs["edge_index"])
    K, src32, em, emt, dbidx = _host_prep(edge_index)
    b1f = np.asarray(inputs["b1"], np.float32)
    b2f = np.asarray(inputs["b2"], np.float32)
    zero_bias = not (np.any(b1f) or np.any(b2f))
    nc = _get_program(K, zero_bias)

    xpad = np.zeros((NPAD, IN_C + 4), np.float32)
    xpad[:N, :IN_C] = x
    X516 = np.ascontiguousarray(xpad).astype(ml_dtypes.bfloat16)
    XT = np.ascontiguousarray(xpad[:, :IN_C].T).astype(ml_dtypes.bfloat16)
    W1f = np.asarray(inputs["W1"], np.float32)
    W2f = np.asarray(inputs["W2"], np.float32)
    WT12 = np.ascontiguousarray(
        np.concatenate([W1f.T, W2f.T], axis=0)).astype(ml_dtypes.bfloat16)
    W12 = np.ascontiguousarray(
        np.concatenate([W1f, W2f], axis=0)).astype(ml_dtypes.bfloat16)
    A4 = np.stack(
        [np.asarray(inputs["a_src1"], np.float32),
         np.asarray(inputs["a_dst1"], np.float32),
         np.asarray(inputs["a_src2"], np.float32),
         np.asarray(inputs["a_dst2"], np.float32)],
        axis=1)  # [1024, 4]
    A4R = np.ascontiguousarray(
        A4.reshape(8, 128, 4).transpose(1, 0, 2).reshape(128, 32)).astype(
            ml_dtypes.bfloat16)
    B12 = np.ascontiguousarray(np.stack(
        [np.asarray(inputs["b1"], np.float32),
         np.asarray(inputs["b2"], np.float32)], axis=0))
    # Wp1 in fp8 DoubleRow-interleaved layout:
    # WP1DR[k2*128+p, t*1024+o2*128+m] = Wp1[(2*k2+t)*128+p, o2*128+m]
    Wp1f = np.asarray(inputs["Wp1"], np.float32).reshape(4, 2, P, 8, P)
    WP1DR = np.ascontiguousarray(
        Wp1f.transpose(0, 2, 1, 3, 4).reshape(4 * P, 2 * OUT_C)).astype(
            ml_dtypes.float8_e4m3)
    base = {
        "X516": X516, "XT": XT, "WT12": WT12, "A4R": A4R, "W12": W12,
        "WP1DR": WP1DR,
        "WP2": np.ascontiguousarray(
            np.asarray(inputs["Wp2"], np.float32)).astype(ml_dtypes.bfloat16),
        "B12": B12,
        "BP1C": np.ascontiguousarray(
            np.asarray(inputs["bp1"], np.float32).reshape(8, P).T),
        "PRA": np.asarray(inputs["prelu_a"], np.float32).reshape(1, 1),
    }
    in_maps = []
    for c in range(NCORES):
        m = dict(base)
        m["MSK"] = np.array([[0.0 if c == NCORES - 1 else 1.0]], np.float32)
        m["EMB"] = np.ascontiguousarray(em[c]).astype(ml_dtypes.bfloat16)
        m["EMTB"] = np.ascontiguousarray(emt[c]).astype(ml_dtypes.bfloat16)
        m["SRC32"] = np.ascontiguousarray(src32[c])
        m["DBIDX"] = np.ascontiguousarray(dbidx[c])
        in_maps.append(m)

    if trace:
        trace = _ensure_trace_support()
    res = bass_utils.run_bass_kernel_spmd(
        nc, in_maps, core_ids=list(range(NCORES)), trace=trace,
        tmpdir=tmpdir)
    out = np.concatenate([res.results[c]["OUT"] for c in range(NCORES)],
                         axis=0)[:N]
    return out, res.exec_time_ns


def kernel(**inputs):
    out, _ = _run(inputs, trace=False)
    return out

